# revision 7
# baseline (speedup 1.0000x reference)
"""Trainium2 Bass kernel for nn_DSnetwork (gnn_message_passing).

Reference computation (S=131072 subgraphs, G=4096 graphs, N=2M nodes, D=128):
  h_sub  = segment_mean(h_node, subgraph_batch, S)            # [S,128]
  2x DS layers:
    x1 = h_sub @ W + b
    x2 = segment_mean(h_sub, subgraph_idx_batch, G) @ Ws + bs
    h_sub = elu(x1 + x2[subgraph_idx_batch])
  h_graph = segment_mean(h_sub, subgraph_idx_batch, G)
  out = relu(h_graph @ Wf1 + bf1) @ Wf2 + bf2                 # [G,10]

Distribution: data-parallel over contiguous graph ranges (512 graphs per
core, 8 cores).  Indices are sorted, so each core owns contiguous slices
of subgraphs and nodes.  Segment sums run on TensorE as one-hot matmuls
(one-hots built on VectorE from host-precomputed relative ids); the
graph->subgraph broadcast uses a DMA row gather.  Matmuls are bf16 with
fp32 PSUM accumulation; mean scaling is exact fp32 on ScalarE.

Host-side work is pure index preprocessing and data staging: sharding,
padded placement of subgraphs/nodes into static tiles, relative one-hot
ids, 1/count scale vectors, and dtype casts.
"""

from dataclasses import dataclass

import ml_dtypes
import numpy as np

BF16 = ml_dtypes.bfloat16
P = 128


@dataclass(frozen=True)
class Cfg:
    D: int = 128          # node feature dim
    C: int = 128          # hidden dim
    NCORES: int = 8
    G_SH: int = 512       # graphs per core
    NGC: int = 4          # graph chunks of 128 graphs per core
    T2: int = 36          # seg tiles (128 segs) per graph chunk, padded
    T1: int = 18          # node tile slots (128 nodes) per seg tile
    SWATH: int = 4        # seg tiles per elu swath (must divide T2)
    GMAX: int = 3         # max seg tiles per dma_gather call

    @property
    def NS(self):         # seg tiles per core (padded axis)
        return self.NGC * self.T2

    @property
    def SMAXP(self):      # padded segs per core
        return self.NS * P

    @property
    def NSLOT(self):      # node tile slots per core
        return self.NS * self.T1


FULL = Cfg()

# ---------------------------------------------------------------------------
# host-side planner: shard + metadata layout
# ---------------------------------------------------------------------------


def _plan_core(cfg, core, h_node_bf16, sb, sib, seg_cnt, g_cnt):
    g0 = core * cfg.G_SH

    hp = np.zeros((cfg.NSLOT * P, cfg.D), dtype=BF16)
    rel = np.full((P, cfg.NSLOT), -1.0, dtype=np.float32)
    invs = np.zeros((P, cfg.NS), dtype=np.float32)
    rel2 = np.full((P, cfg.NS), -1.0, dtype=np.float32)
    invg = np.zeros((P, cfg.NGC), dtype=np.float32)

    gid_pad = np.zeros(cfg.SMAXP, dtype=np.int64)  # local graph id per padded seg

    for gc in range(cfg.NGC):
        glo = g0 + gc * P
        ghi = glo + P
        a = int(np.searchsorted(sib, glo))
        b = int(np.searchsorted(sib, ghi))
        nseg = b - a
        assert nseg <= cfg.T2 * P, f"T2 too small: {nseg} > {cfg.T2 * P}"
        base_tile = gc * cfg.T2          # first seg tile of this graph chunk

        gl = (sib[a:b] - glo).astype(np.int64)          # in [0,128)
        pad_pos = base_tile * P
        gid_pad[pad_pos:pad_pos + nseg] = gl + gc * P

        r2 = np.full(cfg.T2 * P, -1.0, dtype=np.float32)
        r2[:nseg] = gl.astype(np.float32)
        rel2[:, base_tile:base_tile + cfg.T2] = r2.reshape(cfg.T2, P).T

        ivs = np.zeros(cfg.T2 * P, dtype=np.float32)
        ivs[:nseg] = 1.0 / np.maximum(seg_cnt[a:b], 1).astype(np.float32)
        invs[:, base_tile:base_tile + cfg.T2] = ivs.reshape(cfg.T2, P).T

        invg[:, gc] = 1.0 / np.maximum(g_cnt[glo:ghi], 1).astype(np.float32)

        seg_starts = np.searchsorted(sb, np.arange(a, b + 1))
        for t2 in range(cfg.T2):
            k = base_tile + t2                       # seg tile index
            slo = t2 * P
            shi = min(slo + P, nseg)
            if slo >= nseg:
                continue
            nlo = int(seg_starts[slo])
            nhi = int(seg_starts[shi])
            nn = nhi - nlo
            assert nn <= cfg.T1 * P, f"T1 too small: {nn} > {cfg.T1 * P}"
            if nn == 0:
                continue
            dst = k * cfg.T1 * P
            hp[dst:dst + nn] = h_node_bf16[nlo:nhi]
            rr = (sb[nlo:nhi] - (a + slo)).astype(np.float32)
            rfull = np.full(cfg.T1 * P, -1.0, dtype=np.float32)
            rfull[:nn] = rr
            rel[:, k * cfg.T1:(k + 1) * cfg.T1] = rfull.reshape(cfg.T1, P).T

    # gather indices: wrapped by 16, replicated across the 8 gpsimd cores
    gidx = np.tile(
        gid_pad.astype(np.int16).reshape(cfg.SMAXP // 16, 16).T, (8, 1))

    # device layout: hp_dram[k, p, t*D+d] = node row (k*T1*P + t*P + p)
    hp_dev = np.ascontiguousarray(
        hp.reshape(cfg.NS, cfg.T1, P, cfg.D).transpose(0, 2, 1, 3)
    ).reshape(cfg.NS, P, cfg.T1 * cfg.D)
    return {
        "hp": hp_dev,
        "rel": rel,
        "invs": invs,
        "rel2": rel2,
        "invg": invg,
        "gidx": gidx,
    }


def plan(cfg, h_node, sb, sib):
    sb = np.asarray(sb).astype(np.int64)
    sib = np.asarray(sib).astype(np.int64)
    S = sib.shape[0]
    G = cfg.NCORES * cfg.G_SH
    seg_cnt = np.bincount(sb, minlength=S)
    g_cnt = np.bincount(sib, minlength=G)
    h_bf16 = np.asarray(h_node).astype(BF16)
    return [
        _plan_core(cfg, c, h_bf16, sb, sib, seg_cnt, g_cnt)
        for c in range(cfg.NCORES)
    ]


# ---------------------------------------------------------------------------
# bass program
# ---------------------------------------------------------------------------


def build_bass(cfg):
    import concourse.mybir as mybir
    import concourse.tile as tile
    from concourse import bacc

    f32 = mybir.dt.float32
    bf16 = mybir.dt.bfloat16
    i16 = mybir.dt.int16
    AF = mybir.ActivationFunctionType
    OP = mybir.AluOpType
    D, C = cfg.D, cfg.C

    nc = bacc.Bacc("TRN2", target_bir_lowering=False, debug=False)

    def din(name, shape, dt=f32):
        return nc.dram_tensor(name, shape, dt, kind="ExternalInput").ap()

    hp_d = din("hp", [cfg.NS, P, cfg.T1 * D], bf16)
    rel_d = din("rel", [P, cfg.NSLOT])
    invs_d = din("invs", [P, cfg.NS])
    rel2_d = din("rel2", [P, cfg.NS])
    invg_d = din("invg", [P, cfg.NGC])
    gidx_d = din("gidx", [P, cfg.SMAXP // 16], i16)
    iota_d = din("iota", [P, P])
    ident_d = din("ident", [P, P], bf16)

    w_d = {}
    for l in range(2):
        w_d[f"W{l}"] = din(f"W{l}", [D, C])
        w_d[f"Ws{l}"] = din(f"Ws{l}", [D, C])
        w_d[f"b{l}"] = din(f"b{l}", [C])
        w_d[f"bs{l}"] = din(f"bs{l}", [C])
    w_d["Wf1"] = din("Wf1", [C, 2 * C])
    w_d["bf1"] = din("bf1", [2 * C])
    w_d["Wf2"] = din("Wf2", [2 * C, 10])
    w_d["bf2"] = din("bf2", [10])

    out_d = nc.dram_tensor("out", [10, cfg.G_SH], f32, kind="ExternalOutput").ap()
    x2_d = [nc.dram_tensor(f"x2scratch{l}", [cfg.G_SH, C], bf16).ap()
            for l in range(2)]

    with tile.TileContext(nc) as tc:
        with (
            tc.tile_pool(name="persist", bufs=1) as pp,
            tc.tile_pool(name="stream", bufs=2) as sp,
            tc.tile_pool(name="small", bufs=2) as mp,
            tc.tile_pool(name="psum_acc", bufs=4, space="PSUM") as pacc,
            tc.tile_pool(name="psum_tr", bufs=2, space="PSUM") as ptr,
            tc.tile_pool(name="psum_wide", bufs=2, space="PSUM") as pwide,
        ):
            # ---- constants / weights to SBUF -------------------------------
            def load(ap_dram, shape, dt):
                t = pp.tile(shape, dt, tag=f"ld_{ap_dram.tensor.name}")
                nc.sync.dma_start(t[:], ap_dram)
                return t

            iota = load(iota_d, [P, P], f32)
            ident = load(ident_d, [P, P], bf16)
            rel = load(rel_d, [P, cfg.NSLOT], f32)
            invs = load(invs_d, [P, cfg.NS], f32)
            rel2 = load(rel2_d, [P, cfg.NS], f32)
            invg = load(invg_d, [P, cfg.NGC], f32)
            gidx = load(gidx_d, [P, cfg.SMAXP // 16], i16)

            def cast_bf16(name, tf, shape):
                tb = pp.tile(shape, bf16, tag=f"bf_{name}")
                nc.vector.tensor_copy(tb[:], tf[:])
                return tb

            W, Ws, bsum = [], [], []
            for l in range(2):
                W.append(cast_bf16(
                    f"W{l}", load(w_d[f"W{l}"], [D, C], f32), [D, C]))
                Ws.append(cast_bf16(
                    f"Ws{l}", load(w_d[f"Ws{l}"], [D, C], f32), [D, C]))
                b_t = load(w_d[f"b{l}"].unsqueeze(1), [P, 1], f32)
                bs_t = load(w_d[f"bs{l}"].unsqueeze(1), [P, 1], f32)
                s = pp.tile([P, 1], f32, tag=f"bsum{l}")
                nc.vector.tensor_tensor(s[:], b_t[:], bs_t[:], op=OP.add)
                bsum.append(s)
            Wf1 = cast_bf16("Wf1", load(w_d["Wf1"], [C, 2 * C], f32),
                            [C, 2 * C])
            Wf2 = cast_bf16(
                "Wf2",
                load(w_d["Wf2"].rearrange("(h p) t -> p h t", h=2),
                     [P, 2, 10], f32),
                [P, 2, 10])
            bf1 = load(w_d["bf1"].rearrange("(h p) -> p h", h=2), [P, 2], f32)
            bf2_t = pp.tile([P, 1], f32, tag="ld_bf2")
            nc.sync.dma_start(bf2_t[:10, :], w_d["bf2"].unsqueeze(1))

            # persistent activations: per graph chunk [seg_p, (t2, d)]
            hs_a = [pp.tile([P, cfg.T2, D], bf16, tag=f"hsa{gc}", name=f"hsa{gc}")
                    for gc in range(cfg.NGC)]
            hs_b = [pp.tile([P, cfg.T2, D], bf16, tag=f"hsb{gc}", name=f"hsb{gc}")
                    for gc in range(cfg.NGC)]
            ohg = [pp.tile([P, cfg.T2, P], bf16, tag=f"ohg{gc}", name=f"ohg{gc}")
                   for gc in range(cfg.NGC)]

            # ---- graph-level one-hots (built once, reused) -----------------
            for gc in range(cfg.NGC):
                r2b = rel2[:, gc * cfg.T2:(gc + 1) * cfg.T2] \
                    .unsqueeze(2).to_broadcast([P, cfg.T2, P])
                iob = iota[:].unsqueeze(1).to_broadcast([P, cfg.T2, P])
                nc.vector.tensor_tensor(ohg[gc][:], r2b, iob, op=OP.is_equal)

            # ---- phase 1: node -> subgraph mean ----------------------------
            for gc in range(cfg.NGC):
                for t2 in range(cfg.T2):
                    k = gc * cfg.T2 + t2
                    hpt = sp.tile([P, cfg.T1 * D], bf16, tag="hp", bufs=3)
                    nc.sync.dma_start(hpt[:], hp_d[k])
                    oh = sp.tile([P, cfg.T1, P], bf16, tag="oh")
                    rb = rel[:, k * cfg.T1:(k + 1) * cfg.T1] \
                        .unsqueeze(2).to_broadcast([P, cfg.T1, P])
                    iob = iota[:].unsqueeze(1).to_broadcast([P, cfg.T1, P])
                    nc.vector.tensor_tensor(oh[:], rb, iob, op=OP.is_equal)
                    ps = pacc.tile([P, D], f32, tag="acc")
                    for t in range(cfg.T1):
                        nc.tensor.matmul(
                            ps[:], lhsT=oh[:, t, :],
                            rhs=hpt[:, t * D:(t + 1) * D],
                            start=(t == 0), stop=(t == cfg.T1 - 1))
                    nc.scalar.activation(
                        hs_a[gc][:, t2, :], ps[:], AF.Copy,
                        scale=invs[:, k:k + 1])

            # ---- DS layers -------------------------------------------------
            hs_in, hs_out = hs_a, hs_b
            for l in range(2):
                # graph means -> transposed [d, g] table
                gmT = mp.tile([P, cfg.NGC * P], bf16, tag="gmT")
                for gc in range(cfg.NGC):
                    psg = pacc.tile([P, D], f32, tag="acc")
                    for t2 in range(cfg.T2):
                        nc.tensor.matmul(
                            psg[:], lhsT=ohg[gc][:, t2, :],
                            rhs=hs_in[gc][:, t2, :],
                            start=(t2 == 0), stop=(t2 == cfg.T2 - 1))
                    gm = mp.tile([P, D], bf16, tag="gm")
                    nc.scalar.activation(gm[:], psg[:], AF.Copy,
                                         scale=invg[:, gc:gc + 1])
                    ptt = ptr.tile([P, P], bf16, tag="tr")
                    nc.tensor.transpose(ptt[:], gm[:], ident[:])
                    nc.vector.tensor_copy(gmT[:, gc * P:(gc + 1) * P], ptt[:])

                # x2 = gmean @ Ws + (b + bs), written row-major to DRAM
                x2ps = pwide.tile([P, cfg.NGC * P], f32, tag="wide")
                nc.tensor.matmul(x2ps[:], lhsT=Ws[l][:], rhs=gmT[:],
                                 start=True, stop=True)
                x2T = mp.tile([P, cfg.NGC * P], bf16, tag="x2T")
                nc.scalar.activation(x2T[:], x2ps[:], AF.Identity,
                                     bias=bsum[l][:])
                x2rm = mp.tile([P, cfg.NGC, C], bf16, tag="x2rm")
                for gc in range(cfg.NGC):
                    ptt = ptr.tile([P, P], bf16, tag="tr")
                    nc.tensor.transpose(ptt[:], x2T[:, gc * P:(gc + 1) * P],
                                        ident[:])
                    nc.vector.tensor_copy(x2rm[:, gc, :], ptt[:])
                nc.sync.dma_start(
                    x2_d[l].rearrange("(gc p) c -> p gc c", gc=cfg.NGC),
                    x2rm[:])

                npc = cfg.T2 * P // 16   # gidx columns per graph chunk
                for gc in range(cfg.NGC):
                    gath = sp.tile([P, cfg.T2, C], bf16, tag="gath")
                    for j0 in range(0, cfg.T2, cfg.GMAX):
                        j1 = min(j0 + cfg.GMAX, cfg.T2)
                        c0 = gc * npc + j0 * 8
                        c1 = gc * npc + j1 * 8
                        nc.gpsimd.dma_gather(
                            out_ap=gath[:, j0:j1, :],
                            in_ap=x2_d[l],
                            idxs_ap=gidx[:, c0:c1],
                            num_idxs=(j1 - j0) * P,
                            num_idxs_reg=(j1 - j0) * P,
                            elem_size=C,
                        )
                    for sw in range(cfg.T2 // cfg.SWATH):
                        comb = mp.tile([P, cfg.SWATH, C], f32, tag="comb")
                        for j in range(cfg.SWATH):
                            t2 = sw * cfg.SWATH + j
                            ptt = ptr.tile([P, P], bf16, tag="tr")
                            nc.tensor.transpose(ptt[:], hs_in[gc][:, t2, :],
                                                ident[:])
                            hT = mp.tile([P, P], bf16, tag="hT")
                            nc.vector.tensor_copy(hT[:], ptt[:])
                            x1p = pacc.tile([P, C], f32, tag="acc")
                            nc.tensor.matmul(x1p[:], lhsT=hT[:], rhs=W[l][:],
                                             start=True, stop=True)
                            nc.vector.tensor_tensor(
                                comb[:, j, :], x1p[:], gath[:, t2, :],
                                op=OP.add)
                        # elu(comb) -> hs_out, flattened over the swath
                        cf = comb[:].rearrange("p a b -> p (a b)")
                        F = cfg.SWATH * C
                        neg = mp.tile([P, F], f32, tag="neg")
                        nc.vector.tensor_scalar_min(neg[:], cf, 0.0)
                        ex = mp.tile([P, F], f32, tag="ex")
                        nc.scalar.activation(ex[:], neg[:], AF.Exp)
                        nc.vector.tensor_scalar(
                            cf, cf, 0.0, -1.0, op0=OP.max, op1=OP.add)
                        ho = hs_out[gc][:, sw * cfg.SWATH:(sw + 1) * cfg.SWATH, :]
                        nc.vector.tensor_tensor(
                            ho.rearrange("p a b -> p (a b)"), ex[:], cf,
                            op=OP.add)
                hs_in, hs_out = hs_out, hs_in

            # ---- head ------------------------------------------------------
            hgT = mp.tile([P, cfg.NGC * P], bf16, tag="hgT")
            for gc in range(cfg.NGC):
                psg = pacc.tile([P, D], f32, tag="acc")
                for t2 in range(cfg.T2):
                    nc.tensor.matmul(
                        psg[:], lhsT=ohg[gc][:, t2, :],
                        rhs=hs_in[gc][:, t2, :],
                        start=(t2 == 0), stop=(t2 == cfg.T2 - 1))
                gm = mp.tile([P, D], bf16, tag="gm")
                nc.scalar.activation(gm[:], psg[:], AF.Copy,
                                     scale=invg[:, gc:gc + 1])
                ptt = ptr.tile([P, P], bf16, tag="tr")
                nc.tensor.transpose(ptt[:], gm[:], ident[:])
                nc.vector.tensor_copy(hgT[:, gc * P:(gc + 1) * P], ptt[:])

            y1 = []
            for h in range(2):
                yps = pwide.tile([P, cfg.NGC * P], f32, tag="wide")
                nc.tensor.matmul(yps[:], lhsT=Wf1[:, h * C:(h + 1) * C],
                                 rhs=hgT[:], start=True, stop=True)
                y1t = mp.tile([P, cfg.NGC * P], bf16, tag=f"y1_{h}")
                nc.scalar.activation(y1t[:], yps[:], AF.Relu,
                                     bias=bf1[:, h:h + 1])
                y1.append(y1t)
            y2ps = pwide.tile([P, cfg.NGC * P], f32, tag="wide")
            for h in range(2):
                nc.tensor.matmul(y2ps[:10, :], lhsT=Wf2[:, h, :],
                                 rhs=y1[h][:], start=(h == 0), stop=(h == 1))
            yout = mp.tile([P, cfg.NGC * P], f32, tag="yout")
            nc.scalar.activation(yout[:10, :], y2ps[:10, :], AF.Identity,
                                 bias=bf2_t[:10, :])
            nc.sync.dma_start(out_d[:], yout[:10, :])

    nc.compile()
    return nc


# ---------------------------------------------------------------------------
# entry point
# ---------------------------------------------------------------------------

_CACHED = {}


def _get_nc(cfg):
    key = (cfg.T1, cfg.T2, cfg.NGC, cfg.G_SH, cfg.NCORES)
    if key not in _CACHED:
        _CACHED[key] = build_bass(cfg)
    return _CACHED[key]


def make_in_maps(cfg, inputs):
    plans = plan(cfg, inputs["h_node"], inputs["subgraph_batch"],
                 inputs["subgraph_idx_batch"])
    iota = np.broadcast_to(
        np.arange(P, dtype=np.float32), (P, P)).copy()
    ident = np.eye(P, dtype=BF16)
    shared = {
        "iota": iota,
        "ident": ident,
        "W0": np.asarray(inputs["W_fc0"], np.float32),
        "Ws0": np.asarray(inputs["W_sum0"], np.float32),
        "b0": np.asarray(inputs["b_fc0"], np.float32),
        "bs0": np.asarray(inputs["b_sum0"], np.float32),
        "W1": np.asarray(inputs["W_fc1"], np.float32),
        "Ws1": np.asarray(inputs["W_sum1"], np.float32),
        "b1": np.asarray(inputs["b_fc1"], np.float32),
        "bs1": np.asarray(inputs["b_sum1"], np.float32),
        "Wf1": np.asarray(inputs["Wf1"], np.float32),
        "bf1": np.asarray(inputs["bf1"], np.float32),
        "Wf2": np.asarray(inputs["Wf2"], np.float32),
        "bf2": np.asarray(inputs["bf2"], np.float32),
    }
    return [dict(shared, **p) for p in plans]


def run(cfg, inputs, trace=False):
    from concourse.bass_utils import run_bass_kernel_spmd

    in_maps = make_in_maps(cfg, inputs)
    nc = _get_nc(cfg)
    res = run_bass_kernel_spmd(nc, in_maps, list(range(cfg.NCORES)),
                               trace=trace)
    outs = [np.asarray(res.results[c]["out"]).T for c in range(cfg.NCORES)]
    out = np.concatenate(outs, axis=0).astype(np.float32)
    return out, res


def kernel(**inputs) -> np.ndarray:
    out, _ = run(FULL, inputs)
    return out
